# Initial kernel scaffold
#
"""Trainium2 Bass kernel for nn_BoxHead: 2-layer MLP (12544->1024->1024) + two
linear heads (1024->4 classes, 1024->12 box deltas) over 16384 proposals.

Strategy (8 NeuronCores, data-parallel over proposals):
  - Host shards feature_vectors rows 8 ways and pre-transposes each shard to
    [FEAT, rows] so the contraction dim lands on SBUF partitions (no on-chip
    transposes needed). Weights are replicated; Wc/Wr are fused into one
    [1024, 16] head matrix.
  - All matmuls run in float32r (fp32 with 11-bit RNE mantissa rounding done
    by the PE on ingest) -> full 1 cycle/row tensor-engine rate with ~1e-4
    relative error; accumulation stays fp32 in PSUM.
  - Everything is computed transposed: h1T/h2T are [1024, rows_per_core] and
    live in SBUF; the only DRAM round trips are the raw inputs and the tiny
    [16, rows] output per core.
  - Layer 1 streams W1 exactly once (51 MB) using k-groups of G=14 k-tiles
    accumulated in PSUM, then added into an SBUF fp32 accumulator (DVE), so
    total DMA per core is ~158 MB vs ~730 us of PE work -> compute bound.

kernel(**inputs) takes the full unsharded inputs and returns
(class_logits [16384,4], box_pred [16384,12]) as float32, matching reference.
"""

import sys

if "/opt/trn_rl_repo" not in sys.path:
    sys.path.insert(0, "/opt/trn_rl_repo")

import numpy as np
from contextlib import ExitStack

N_CORES = 8
N_PROP = 16384
ROWS = N_PROP // N_CORES   # 2048 rows per core
FEAT = 12544
HID = 1024
KT1 = FEAT // 128          # 98 k-tiles in layer 1
G = 14                     # k-tiles per group (98 = 7 * 14)
NG = KT1 // G              # 7 groups
MT = HID // 128            # 8 m-tiles
KT2 = HID // 128           # 8 k-tiles in layers 2/3
NOUT = 16                  # 4 class logits + 12 box regression, fused
RQ = 4                     # row quarters
RQW = ROWS // RQ           # 512 (= max fp32 matmul free dim = one PSUM bank)

_NC = None


def _build():
    import concourse.tile as tile
    from concourse import bacc, mybir

    f32 = mybir.dt.float32
    f32r = mybir.dt.float32r
    Relu = mybir.ActivationFunctionType.Relu
    Identity = mybir.ActivationFunctionType.Identity

    nc = bacc.Bacc("TRN2", target_bir_lowering=False, debug=False, num_devices=N_CORES)

    xt_ap = nc.dram_tensor("xt", [FEAT, ROWS], f32r, kind="ExternalInput").ap()
    w1_ap = nc.dram_tensor("w1", [FEAT, HID], f32r, kind="ExternalInput").ap()
    w2_ap = nc.dram_tensor("w2", [HID, HID], f32r, kind="ExternalInput").ap()
    wcr_ap = nc.dram_tensor("wcr", [HID, NOUT], f32r, kind="ExternalInput").ap()
    b1_ap = nc.dram_tensor("b1", [128, MT], f32, kind="ExternalInput").ap()
    b2_ap = nc.dram_tensor("b2", [128, MT], f32, kind="ExternalInput").ap()
    bcr_ap = nc.dram_tensor("bcr", [NOUT, 1], f32, kind="ExternalInput").ap()
    out_ap = nc.dram_tensor("out", [NOUT, ROWS], f32, kind="ExternalOutput").ap()

    with tile.TileContext(nc) as tc:
        with ExitStack() as ctx:
            const = ctx.enter_context(tc.tile_pool(name="const", bufs=1))
            b1_t = const.tile([128, MT], f32)
            nc.sync.dma_start(b1_t[:], b1_ap[:, :])
            b2_t = const.tile([128, MT], f32)
            nc.sync.dma_start(b2_t[:], b2_ap[:, :])
            bcr_t = const.tile([NOUT, 1], f32)
            nc.sync.dma_start(bcr_t[:], bcr_ap[:, :])

            # h1T survives into phase 2; allocated on the outer stack.
            h1_pool = ctx.enter_context(tc.tile_pool(name="h1", bufs=1))
            h1 = h1_pool.tile([128, MT, ROWS], f32r)

            # ---- Phase 1: acc = W1^T @ XT (fp32 acc in SBUF), h1 = relu(acc+b1)
            with ExitStack() as ctxA:
                acc_pool = ctxA.enter_context(tc.tile_pool(name="acc", bufs=1))
                acc = acc_pool.tile([128, MT, ROWS], f32)
                with ExitStack() as ctx1:
                    w1p = ctx1.enter_context(tc.tile_pool(name="w1p", bufs=G + 2))
                    xtp = ctx1.enter_context(tc.tile_pool(name="xtp", bufs=G + 4))
                    psp = ctx1.enter_context(tc.tile_pool(name="ps1", bufs=6, space="PSUM"))
                    for g in range(NG):
                        ks = list(range(g * G, (g + 1) * G))
                        w1_t = {}
                        for k in ks:
                            w1_t[k] = w1p.tile([128, HID], f32r, tag="w1")
                            nc.sync.dma_start(w1_t[k][:], w1_ap[k * 128:(k + 1) * 128, :])
                        for rq in range(RQ):
                            xt_t = {}
                            for k in ks:
                                xt_t[k] = xtp.tile([128, RQW], f32r, tag="xt")
                                nc.sync.dma_start(
                                    xt_t[k][:],
                                    xt_ap[k * 128:(k + 1) * 128, rq * RQW:(rq + 1) * RQW],
                                )
                            for m in range(MT):
                                ps = psp.tile([128, RQW], f32)
                                for i, k in enumerate(ks):
                                    nc.tensor.matmul(
                                        ps[:],
                                        w1_t[k][:, m * 128:(m + 1) * 128],
                                        xt_t[k][:],
                                        start=(i == 0),
                                        stop=(i == G - 1),
                                    )
                                dst = acc[:, m, rq * RQW:(rq + 1) * RQW]
                                if g == 0:
                                    nc.vector.tensor_copy(out=dst, in_=ps[:])
                                else:
                                    nc.vector.tensor_add(dst, dst, ps[:])
                for m in range(MT):
                    nc.scalar.activation(h1[:, m, :], acc[:, m, :], Relu, bias=b1_t[:, m:m + 1])
            # acc freed here

            # ---- Phase 2: h2 = relu(W2^T @ h1 + b2)
            h2_pool = ctx.enter_context(tc.tile_pool(name="h2", bufs=1))
            h2 = h2_pool.tile([128, MT, ROWS], f32r)
            w2p = ctx.enter_context(tc.tile_pool(name="w2p", bufs=1))
            w2_t = w2p.tile([128, KT2, HID], f32r)
            for k in range(KT2):
                nc.sync.dma_start(w2_t[:, k, :], w2_ap[k * 128:(k + 1) * 128, :])
            with ExitStack() as ctx2:
                ps2p = ctx2.enter_context(tc.tile_pool(name="ps2", bufs=2, space="PSUM"))
                for m in range(MT):
                    ps = ps2p.tile([128, ROWS], f32)
                    for k in range(KT2):
                        for rq in range(RQ):
                            nc.tensor.matmul(
                                ps[:, rq * RQW:(rq + 1) * RQW],
                                w2_t[:, k, m * 128:(m + 1) * 128],
                                h1[:, k, rq * RQW:(rq + 1) * RQW],
                                start=(k == 0),
                                stop=(k == KT2 - 1),
                            )
                    nc.scalar.activation(h2[:, m, :], ps[:], Relu, bias=b2_t[:, m:m + 1])

            # ---- Phase 3: out = Wcr^T @ h2 + bcr   ([16, ROWS])
            wcrp = ctx.enter_context(tc.tile_pool(name="wcrp", bufs=1))
            wcr_t = wcrp.tile([128, KT2, NOUT], f32r)
            for k in range(KT2):
                nc.sync.dma_start(wcr_t[:, k, :], wcr_ap[k * 128:(k + 1) * 128, :])
            outp = ctx.enter_context(tc.tile_pool(name="outp", bufs=1))
            out_t = outp.tile([NOUT, ROWS], f32)
            with ExitStack() as ctx3:
                ps3p = ctx3.enter_context(tc.tile_pool(name="ps3", bufs=1, space="PSUM"))
                ps3 = ps3p.tile([NOUT, ROWS], f32)
                for k in range(KT2):
                    for rq in range(RQ):
                        nc.tensor.matmul(
                            ps3[:, rq * RQW:(rq + 1) * RQW],
                            wcr_t[:, k, :],
                            h2[:, k, rq * RQW:(rq + 1) * RQW],
                            start=(k == 0),
                            stop=(k == KT2 - 1),
                        )
                nc.scalar.activation(out_t[:], ps3[:], Identity, bias=bcr_t[:])
            nc.sync.dma_start(out_ap[:, :], out_t[:])

    nc.compile()
    return nc


def _get_nc():
    global _NC
    if _NC is None:
        _NC = _build()
    return _NC


def _prepare_in_maps(feature_vectors, W1, b1, W2, b2, Wc, bc, Wr, br):
    fv = np.asarray(feature_vectors, dtype=np.float32)
    w1 = np.ascontiguousarray(np.asarray(W1, dtype=np.float32))
    w2 = np.ascontiguousarray(np.asarray(W2, dtype=np.float32))
    wcr = np.ascontiguousarray(
        np.concatenate(
            [np.asarray(Wc, dtype=np.float32), np.asarray(Wr, dtype=np.float32)], axis=1
        )
    )
    b1m = np.ascontiguousarray(np.asarray(b1, dtype=np.float32).reshape(MT, 128).T)
    b2m = np.ascontiguousarray(np.asarray(b2, dtype=np.float32).reshape(MT, 128).T)
    bcr = np.ascontiguousarray(
        np.concatenate([np.asarray(bc, dtype=np.float32), np.asarray(br, dtype=np.float32)]).reshape(NOUT, 1)
    )
    in_maps = []
    for i in range(N_CORES):
        xt = np.ascontiguousarray(fv[i * ROWS:(i + 1) * ROWS, :].T)
        in_maps.append(
            {"xt": xt, "w1": w1, "w2": w2, "wcr": wcr, "b1": b1m, "b2": b2m, "bcr": bcr}
        )
    return in_maps


def _assemble(results):
    logits = np.concatenate([r["out"][:4, :].T for r in results], axis=0)
    box = np.concatenate([r["out"][4:, :].T for r in results], axis=0)
    return np.ascontiguousarray(logits), np.ascontiguousarray(box)


def _execute(in_maps, trace=False, **kwargs):
    from concourse import bass_utils

    nc = _get_nc()
    return bass_utils.run_bass_kernel_spmd(
        nc, in_maps, core_ids=list(range(N_CORES)), trace=trace, **kwargs
    )


def kernel(feature_vectors, W1, b1, W2, b2, Wc, bc, Wr, br):
    in_maps = _prepare_in_maps(feature_vectors, W1, b1, W2, b2, Wc, bc, Wr, br)
    res = _execute(in_maps, trace=False)
    return _assemble(res.results)


# revision 8
# speedup vs baseline: 1.3286x; 1.3286x over previous
"""Trainium2 Bass kernel for nn_BoxHead: 2-layer MLP (12544->1024->1024) + two
linear heads (1024->4 classes, 1024->12 box deltas) over 16384 proposals.

Strategy (8 NeuronCores, data-parallel over proposals):
  - Host shards feature_vectors rows 8 ways and pre-transposes each shard to
    [FEAT, rows] so the contraction dim lands on SBUF partitions (no on-chip
    transposes needed). Weights are replicated; Wc/Wr are fused into one
    [1024, 16] head matrix.
  - All matmuls run in float32r (fp32 with 11-bit RNE mantissa rounding done
    by the PE on ingest) -> full 1 cycle/row tensor-engine rate with ~1e-4
    relative error; accumulation stays fp32 in PSUM.
  - Everything is computed transposed: h1T/h2T are [1024, rows_per_core] and
    live in SBUF; the only DRAM round trips are the raw inputs and the tiny
    [16, rows] output per core.
  - Layer 1 streams W1 exactly once (51 MB) using k-groups of G=14 k-tiles
    accumulated in PSUM, then added into an SBUF fp32 accumulator (DVE), so
    total DMA per core is ~158 MB vs ~730 us of PE work -> compute bound.

kernel(**inputs) takes the full unsharded inputs and returns
(class_logits [16384,4], box_pred [16384,12]) as float32, matching reference.
"""

import sys

if "/opt/trn_rl_repo" not in sys.path:
    sys.path.insert(0, "/opt/trn_rl_repo")

import numpy as np
from contextlib import ExitStack

N_CORES = 8
N_PROP = 16384
ROWS = N_PROP // N_CORES   # 2048 rows per core
FEAT = 12544
HID = 1024
KT1 = FEAT // 128          # 98 k-tiles in layer 1
G = 14                     # k-tiles per group (98 = 7 * 14)
NG = KT1 // G              # 7 groups
MT = HID // 128            # 8 m-tiles
KT2 = HID // 128           # 8 k-tiles in layers 2/3
NOUT = 16                  # 4 class logits + 12 box regression, fused
RQ = 4                     # row quarters
RQW = ROWS // RQ           # 512 (= max fp32 matmul free dim = one PSUM bank)

_NC = None


def _build(reps=1):
    import concourse.tile as tile
    from concourse import bacc, mybir

    f32 = mybir.dt.float32
    f32r = mybir.dt.float32r
    Relu = mybir.ActivationFunctionType.Relu
    Identity = mybir.ActivationFunctionType.Identity

    nc = bacc.Bacc("TRN2", target_bir_lowering=False, debug=False, num_devices=N_CORES)

    xt_ap = nc.dram_tensor("xt", [FEAT, ROWS], f32r, kind="ExternalInput").ap()
    w1_ap = nc.dram_tensor("w1", [FEAT, HID], f32r, kind="ExternalInput").ap()
    w2_ap = nc.dram_tensor("w2", [HID, HID], f32r, kind="ExternalInput").ap()
    wcr_ap = nc.dram_tensor("wcr", [HID, NOUT], f32r, kind="ExternalInput").ap()
    b1_ap = nc.dram_tensor("b1", [128, MT], f32, kind="ExternalInput").ap()
    b2_ap = nc.dram_tensor("b2", [128, MT], f32, kind="ExternalInput").ap()
    bcr_ap = nc.dram_tensor("bcr", [NOUT, 1], f32, kind="ExternalInput").ap()
    out_ap = nc.dram_tensor("out", [NOUT, ROWS], f32, kind="ExternalOutput").ap()

    with tile.TileContext(nc) as tc:
        for _rep in range(reps):
            _build_body(tc, nc, mybir, f32, f32r, Relu, Identity,
                        xt_ap, w1_ap, w2_ap, wcr_ap, b1_ap, b2_ap, bcr_ap, out_ap)

    nc.compile()
    return nc


def _build_body(tc, nc, mybir, f32, f32r, Relu, Identity,
                xt_ap, w1_ap, w2_ap, wcr_ap, b1_ap, b2_ap, bcr_ap, out_ap):
    if True:
        with ExitStack() as ctx:
            const = ctx.enter_context(tc.tile_pool(name="const", bufs=1))
            b1_t = const.tile([128, MT], f32)
            nc.sync.dma_start(b1_t[:], b1_ap[:, :])
            b2_t = const.tile([128, MT], f32)
            nc.sync.dma_start(b2_t[:], b2_ap[:, :])
            bcr_t = const.tile([NOUT, 1], f32)
            nc.sync.dma_start(bcr_t[:], bcr_ap[:, :])

            # ---- Phase 1: acc = W1^T @ XT (fp32 acc in SBUF), h1 = relu(acc+b1)
            with ExitStack() as ctxA:
                acc_pool = ctxA.enter_context(tc.tile_pool(name="acc", bufs=1, side="right"))
                acc = acc_pool.tile([128, MT, ROWS], f32)
                with ExitStack() as ctx1:
                    w1p = ctx1.enter_context(tc.tile_pool(name="w1p", bufs=G + 2, side="right"))
                    xtp = ctx1.enter_context(tc.tile_pool(name="xtp", bufs=G + 4, side="right"))
                    psp = ctx1.enter_context(tc.tile_pool(name="ps1", bufs=6, space="PSUM"))
                    for g in range(NG):
                        ks = list(range(g * G, (g + 1) * G))
                        w1_t = {}
                        for k in ks:
                            w1_t[k] = w1p.tile([128, HID], f32r, tag="w1", name=f"w1_{k}")
                            nc.sync.dma_start(w1_t[k][:], w1_ap[k * 128:(k + 1) * 128, :])
                        for rq in range(RQ):
                            xt_t = {}
                            for k in ks:
                                xt_t[k] = xtp.tile([128, RQW], f32r, tag="xt", name=f"xt_{g}_{rq}_{k}")
                                nc.sync.dma_start(
                                    xt_t[k][:],
                                    xt_ap[k * 128:(k + 1) * 128, rq * RQW:(rq + 1) * RQW],
                                )
                            for m in range(MT):
                                ps = psp.tile([128, RQW], f32)
                                for i, k in enumerate(ks):
                                    nc.tensor.matmul(
                                        ps[:],
                                        w1_t[k][:, m * 128:(m + 1) * 128],
                                        xt_t[k][:],
                                        start=(i == 0),
                                        stop=(i == G - 1),
                                    )
                                dst = acc[:, m, rq * RQW:(rq + 1) * RQW]
                                if g == 0:
                                    nc.vector.tensor_copy(out=dst, in_=ps[:])
                                else:
                                    nc.vector.tensor_add(dst, dst, ps[:])
                # h1T survives into phase 2; opened only after the stream pools
                # above are closed (pools reserve SBUF space at open).
                h1_pool = ctx.enter_context(tc.tile_pool(name="h1", bufs=1))
                h1 = h1_pool.tile([128, MT, ROWS], f32r)
                for m in range(MT):
                    nc.scalar.activation(h1[:, m, :], acc[:, m, :], Relu, bias=b1_t[:, m:m + 1])
            # acc freed here

            # ---- Phase 2: h2 = relu(W2^T @ h1 + b2)
            h2_pool = ctx.enter_context(tc.tile_pool(name="h2", bufs=1))
            h2 = h2_pool.tile([128, MT, ROWS], f32r)
            w2p = ctx.enter_context(tc.tile_pool(name="w2p", bufs=1))
            w2_t = w2p.tile([128, KT2, HID], f32r)
            for k in range(KT2):
                nc.sync.dma_start(w2_t[:, k, :], w2_ap[k * 128:(k + 1) * 128, :])
            with ExitStack() as ctx2:
                ps2p = ctx2.enter_context(tc.tile_pool(name="ps2", bufs=2, space="PSUM"))
                for m in range(MT):
                    ps = ps2p.tile([128, ROWS], f32)
                    for k in range(KT2):
                        for rq in range(RQ):
                            nc.tensor.matmul(
                                ps[:, rq * RQW:(rq + 1) * RQW],
                                w2_t[:, k, m * 128:(m + 1) * 128],
                                h1[:, k, rq * RQW:(rq + 1) * RQW],
                                start=(k == 0),
                                stop=(k == KT2 - 1),
                            )
                    nc.scalar.activation(h2[:, m, :], ps[:], Relu, bias=b2_t[:, m:m + 1])

            # ---- Phase 3: out = Wcr^T @ h2 + bcr   ([16, ROWS])
            wcrp = ctx.enter_context(tc.tile_pool(name="wcrp", bufs=1))
            wcr_t = wcrp.tile([128, KT2, NOUT], f32r)
            for k in range(KT2):
                nc.sync.dma_start(wcr_t[:, k, :], wcr_ap[k * 128:(k + 1) * 128, :])
            outp = ctx.enter_context(tc.tile_pool(name="outp", bufs=1))
            out_t = outp.tile([NOUT, ROWS], f32)
            with ExitStack() as ctx3:
                ps3p = ctx3.enter_context(tc.tile_pool(name="ps3", bufs=1, space="PSUM"))
                ps3 = ps3p.tile([NOUT, ROWS], f32)
                for k in range(KT2):
                    for rq in range(RQ):
                        nc.tensor.matmul(
                            ps3[:, rq * RQW:(rq + 1) * RQW],
                            wcr_t[:, k, :],
                            h2[:, k, rq * RQW:(rq + 1) * RQW],
                            start=(k == 0),
                            stop=(k == KT2 - 1),
                        )
                nc.scalar.activation(out_t[:], ps3[:], Identity, bias=bcr_t[:])
            nc.sync.dma_start(out_ap[:, :], out_t[:])


def _get_nc():
    global _NC
    if _NC is None:
        _NC = _build()
    return _NC


def _prepare_in_maps(feature_vectors, W1, b1, W2, b2, Wc, bc, Wr, br):
    fv = np.asarray(feature_vectors, dtype=np.float32)
    w1 = np.ascontiguousarray(np.asarray(W1, dtype=np.float32))
    w2 = np.ascontiguousarray(np.asarray(W2, dtype=np.float32))
    wcr = np.ascontiguousarray(
        np.concatenate(
            [np.asarray(Wc, dtype=np.float32), np.asarray(Wr, dtype=np.float32)], axis=1
        )
    )
    b1m = np.ascontiguousarray(np.asarray(b1, dtype=np.float32).reshape(MT, 128).T)
    b2m = np.ascontiguousarray(np.asarray(b2, dtype=np.float32).reshape(MT, 128).T)
    bcr = np.ascontiguousarray(
        np.concatenate([np.asarray(bc, dtype=np.float32), np.asarray(br, dtype=np.float32)]).reshape(NOUT, 1)
    )
    in_maps = []
    for i in range(N_CORES):
        xt = np.ascontiguousarray(fv[i * ROWS:(i + 1) * ROWS, :].T)
        in_maps.append(
            {"xt": xt, "w1": w1, "w2": w2, "wcr": wcr, "b1": b1m, "b2": b2m, "bcr": bcr}
        )
    return in_maps


def _assemble(results):
    logits = np.concatenate([r["out"][:4, :].T for r in results], axis=0)
    box = np.concatenate([r["out"][4:, :].T for r in results], axis=0)
    return np.ascontiguousarray(logits), np.ascontiguousarray(box)


def _execute(in_maps, trace=False, **kwargs):
    from concourse import bass_utils

    nc = _get_nc()
    return bass_utils.run_bass_kernel_spmd(
        nc, in_maps, core_ids=list(range(N_CORES)), trace=trace, **kwargs
    )


def kernel(feature_vectors, W1, b1, W2, b2, Wc, bc, Wr, br):
    in_maps = _prepare_in_maps(feature_vectors, W1, b1, W2, b2, Wc, bc, Wr, br)
    res = _execute(in_maps, trace=False)
    return _assemble(res.results)
